# revision 62
# baseline (speedup 1.0000x reference)
"""Trainium2 Bass kernel for nn_CausalSelfAttention_2860448219236 (v5).

Reference semantics (B=2, S=2048, H=1024, NH=16, HD=64, WINDOW=512, NEG=-1e4):
  q/k/v = heads(hs @ W{q,k,v}.T + b)
  mask  = causal(j>i: NEG) + window(j >= i-512: NEG) + attention_mask
  out   = softmax(q k^T/8 + mask) v

Because NEG=-1e4 and softmax subtracts the row max, the f32 result equals a
*binary*-masked softmax over the allowed set
  A(i) = {j <= i}        for i <= 512  (whole row carries the same -1e4)
       = {j <= i-513}    for i >= 513  (recent-window entries underflow to 0)

Sharding: core c = (batch b = c//4) x (head group g = c%4, heads 4g..4g+3).
Fully data-parallel SPMD - one program, per-core input slices, no collectives.

v5 design (cost model: matmul time = out-free-size rows; exp on ACT is the
second-busiest engine, ~49us vs PE ~63us):
  - single interleaved DRAM input `hw`: per k-section the columns are ordered
    by when they are first needed [wqk-e0 | hst-c0 | wqk-e1 | hst-c1 | wv |
    hst-c2 | hst-c3], so startup needs only 4 merged DMAs before the first
    matmul and 13 input DMAs total (each DMA costs ~625ns of serialized HWDGE
    hold + ~2.2us fixed latency).
  - QK of every chunk emitted as early as deps allow so ACT saturates early;
    ALL PV work is deferred and floated to wherever PE has slack vs ACT
    (emit_pv_chunk as feed thunks), ending with a pure-PE endgame that runs
    while ACT drains the last chunk's exps. te tiles stay alive until their
    PV (epool bufs=56 covers every allocation - no recycling).
  - t=512 special pass in column orientation: 20 free-size-1 score matmuls
    into one PSUM bank (sequential start/stop groups per column - data of
    completed columns survives later bank pending-zero marks), ONE 20-wide
    exp, then row-major PV off the column probs. ~0.2us PE + ~0.2us ACT and
    zero DRAM round-trips.
  - scoresT[s,t] grid: QK packs 2 heads via row tile_position; exp on ACT
    with per-partition attn-mask bias; 0/1 diagonal masks on DVE.
  - PV transposed: out[t,d] = te(block)^T @ v -> 65-row matmuls; Z via the
    ones column in v. Normalization per-partition: reciprocal of Z cols +
    broadcast mul; output DRAM layout [t, d].
  - t=512 (the one column whose window boundary is not block-aligned) is
    recomputed exactly by the special pass; the main grid leaves tci=1/tb=4
    col 0 unmasked so Z!=0 and skips storing that row.
"""

import numpy as np

S = 2048
H = 1024
B = 2
NH = 16
HD = 64
SCALE = 0.125
SB = 128          # s block
TC = 512          # t chunk
NTC = S // TC     # 4
NSB = S // SB     # 16
NHC = 4           # heads per core
KTS = S - 512     # kT/v s-extent needed by the main grid (1536)
NVT = KTS // SB   # 12 v tiles
NK = H // SB      # 8 k-tiles
WK = 768          # per-k-tile w cols: q0|k0|q1|k1|v(256)
SEC = WK + S      # interleaved per-k section: 2816 cols
# region offsets inside a k-section: [q0k0 | hst-c0 | wv | q1k1 | hst-c1 |
# hst-c2 | hst-c3] so the pre phase (q00/k00) needs only cols 0:768
_WOFS = (0, 1024, 768)        # wqk-e0, wqk-e1, wv
_HOFS = (256, 1280, 1792, 2304)  # hst chunks 0..3

_CACHE = {}


def _wcol(k, c):
    """w column c (0..WK) of k-tile k -> interleaved hw column."""
    r, o = divmod(c, 256)
    return k * SEC + _WOFS[r] + o


def _hcol(k, t):
    """hst column t (0..S) of k-tile k -> interleaved hw column."""
    cc, o = divmod(t, TC)
    return k * SEC + _HOFS[cc] + o


def _alive_sbs(tci):
    if tci == 0:
        return list(range(4))
    return list(range(min(4 * tci, NVT)))


def _x_lo(sb, tci):
    first_tb = sb if tci == 0 else sb + 4
    return max(0, SB * (first_tb - 4 * tci))


def _sb_last(tb, tci, nsbs):
    # last sb contributing to t-block tb of chunk tci
    if tci == 0:
        return tb
    return min(nsbs - 1, 4 * tci - 4 + tb)


def _diag_actions(sb, tci):
    """[(block_in_chunk, mask_idx, col_off)]; mask 0 = p<=x, 1 = p<=x-1."""
    acts = []
    for tb in range(4 * tci, 4 * tci + 4):
        if tb <= 3 and tb == sb:
            acts.append((tb - 4 * tci, 0, 0))
        if tb >= 4 and tb - 4 == sb:
            # at t0=512 (tci==1, block 0) leave col 0 unmasked: that column
            # (t=512) is recomputed exactly by the special pass, and masking
            # it fully would make Z=0 -> div noise.
            col_off = 1 if (tci == 1 and tb == 4) else 0
            acts.append((tb - 4 * tci, 1, col_off))
    return acts


def _build_program(with_bias=False, with_attc=False):
    import concourse.bass as bass_mod
    import concourse.bacc as bacc
    import concourse.mybir as mybir
    from concourse.tile import TileContext

    F32 = mybir.dt.float32
    BF16 = mybir.dt.bfloat16
    EXP = mybir.ActivationFunctionType.Exp

    nc = bacc.Bacc("TRN2", target_bir_lowering=False, debug=False)

    hw_d = nc.dram_tensor("hw", [SB, NK * SEC], BF16, kind="ExternalInput")
    attc_d = nc.dram_tensor("attc", [SB, NSB], F32, kind="ExternalInput")
    masks_d = nc.dram_tensor("masks", [SB, 2 * SB + 1], BF16,
                             kind="ExternalInput")
    if with_attc:
        attr_d = nc.dram_tensor("attr", [1, 513], F32, kind="ExternalInput")
        attcs_d = nc.dram_tensor("attcs", [SB, NSB], F32,
                                 kind="ExternalInput")
    if with_bias:
        hst9_d = nc.dram_tensor("hst9", [1, S], BF16, kind="ExternalInput")
        w9_d = nc.dram_tensor("w9", [1, WK], BF16, kind="ExternalInput")
    out_d = nc.dram_tensor("out", [S, NHC * HD], F32, kind="ExternalOutput")

    NKA = NK + (1 if with_bias else 0)

    with TileContext(nc) as tc:
        with tc.tile_pool(name="stat", bufs=1) as stat:
            hw = stat.tile([SB, NK * SEC], BF16, tag="hw", name="hw")
            qt = [stat.tile([SB, S], BF16, tag=f"qt{e}", name=f"qt{e}")
                  for e in range(2)]
            kt = [stat.tile([SB, KTS], BF16, tag=f"kt{e}", name=f"kt{e}")
                  for e in range(2)]
            vt = [stat.tile([SB, NHC * (HD + 1)], BF16, tag=f"vt{i}",
                            name=f"vt{i}")
                  for i in range(NVT)]
            attc = stat.tile([SB, NSB], F32, tag="attc", name="attc")
            masks = stat.tile([SB, 2 * SB + 1], BF16, tag="masks",
                              name="masks")
            # special pass (t=512): erT[s, 4g+sb] = probs column-major;
            # cols 16..19 (partition 0 only) hold the j=512 tail per g.
            erT = stat.tile([SB, 20], BF16, tag="erT", name="erT")
            rz4 = stat.tile([1, NHC], F32, tag="rz4", name="rz4")
            svn = stat.tile([1, NHC * HD], F32, tag="svn", name="svn")
            if with_attc:
                attr_t = stat.tile([1, 513], F32, tag="attr", name="attr")
                attcs = stat.tile([SB, NSB], F32, tag="attcs", name="attcs")
            if with_bias:
                hst9 = stat.tile([1, S], BF16, tag="hst9", name="hst9")
                w9 = stat.tile([1, WK], BF16, tag="w9", name="w9")

            # warm the ACT exp table during the DMA-bound startup: the
            # framework inserts the 1.3us LoadActFuncSet before the first
            # activation, so make that a dummy one at t~0


            # warm the ACT exp table during the DMA-bound startup: the
            # framework inserts the 1.3us LoadActFuncSet before the first
            # activation, so make that a dummy one at t~0 (erT cols are
            # overwritten by the real special-pass exp later)
            nc.vector.memset(erT[0:1, 0:4], 0.0)
            nc.scalar.activation(erT[0:1, 4:8], erT[0:1, 0:4], EXP)

            for i in range(NVT):
                ocols = bass_mod.AP(
                    tensor=vt[i][:].tensor, offset=HD,
                    ap=[[NHC * (HD + 1), SB], [HD + 1, NHC], [1, 1]],
                )
                nc.vector.memset(ocols, 1.0)

            # --- merged, need-ordered input DMA (SP queue) ----------------
            def dma_cols(c0, c1, k0, k1):
                src = bass_mod.AP(
                    tensor=hw_d.ap().tensor, offset=k0 * SEC + c0,
                    ap=[[NK * SEC, SB], [SEC, k1 - k0], [1, c1 - c0]],
                )
                dst = bass_mod.AP(
                    tensor=hw[:].tensor, offset=k0 * SEC + c0,
                    ap=[[NK * SEC, SB], [SEC, k1 - k0], [1, c1 - c0]],
                )
                nc.sync.dma_start(out=dst, in_=src)

            # first piece split so the very first matmul (k0, t 0:256) can
            # start ~0.6us sooner
            dma_cols(0, 512, 0, 1)                 # k0: q0k0 + hst t0:256
            dma_cols(512, 768, 0, 1)               # k0: hst t256:512
            for kk in range(1, NK):
                dma_cols(0, 768, kk, kk + 1)       # q0k0 + hst-c0
            dma_cols(768, 1280, 0, 4)              # wv + wqk-e1
            dma_cols(1280, 1792, 0, 4)             # hst-c1
            nc.sync.dma_start(out=masks[:], in_=masks_d[:])
            nc.sync.dma_start(out=attc[:], in_=attc_d[:])
            if with_attc:
                nc.sync.dma_start(out=attr_t[:], in_=attr_d[:])
                nc.sync.dma_start(out=attcs[:], in_=attcs_d[:])
            if with_bias:
                nc.sync.dma_start(out=hst9[:], in_=hst9_d[:])
                nc.sync.dma_start(out=w9[:], in_=w9_d[:])
            dma_cols(768, 1280, 4, NK)
            dma_cols(1280, 1792, 4, NK)
            dma_cols(1792, 2304, 0, 4)             # hst-c2
            dma_cols(1792, 2304, 4, NK)
            dma_cols(2304, 2816, 0, 4)             # hst-c3
            dma_cols(2304, 2816, 4, NK)

            with (
                tc.tile_pool(name="mmps", bufs=2, space="PSUM") as mmps,
                tc.tile_pool(name="ppps", bufs=2, space="PSUM") as ppps,
                tc.tile_pool(name="pvps", bufs=2, space="PSUM") as pvps,
                tc.tile_pool(name="epool", bufs=56) as epool,
                tc.tile_pool(name="rpool", bufs=6) as rpool,
                tc.tile_pool(name="opool", bufs=6) as opool,
            ):

                def thunks_q(e, tcc):
                    box = {}
                    def mk(k):
                        def f():
                            if k == 0:
                                box["pp"] = ppps.tile([SB, TC], F32,
                                                      tag="pp", name="pp")
                            nc.tensor.matmul(
                                box["pp"][:],
                                hw[:, _wcol(k, 256 * e) :
                                   _wcol(k, 256 * e) + SB],
                                hw[:, _hcol(k, tcc * TC) :
                                   _hcol(k, tcc * TC) + TC],
                                start=(k == 0),
                                stop=(k == NKA - 1),
                            )
                            if with_bias and k == NK - 1:
                                nc.tensor.matmul(
                                    box["pp"][:],
                                    w9[:, 256 * e : 256 * e + SB],
                                    hst9[:, tcc * TC : (tcc + 1) * TC],
                                    start=False, stop=True,
                                )
                        return f
                    def cp():
                        nc.vector.tensor_copy(
                            qt[e][:, tcc * TC : (tcc + 1) * TC], box["pp"][:]
                        )
                    return [mk(k) for k in range(NK)] + [cp]

                def thunks_k(e, scc):
                    box = {}
                    off0 = 256 * e + SB
                    def mk(k):
                        def f():
                            if k == 0:
                                box["pp"] = ppps.tile([SB, TC], F32,
                                                      tag="pp", name="pp")
                            nc.tensor.matmul(
                                box["pp"][:],
                                hw[:, _wcol(k, off0) : _wcol(k, off0) + SB],
                                hw[:, _hcol(k, scc * TC) :
                                   _hcol(k, scc * TC) + TC],
                                start=(k == 0),
                                stop=(k == NKA - 1),
                            )
                            if with_bias and k == NK - 1:
                                nc.tensor.matmul(
                                    box["pp"][:],
                                    w9[:, off0 : off0 + SB],
                                    hst9[:, scc * TC : (scc + 1) * TC],
                                    start=False, stop=True,
                                )
                        return f
                    def cp():
                        nc.vector.tensor_copy(
                            kt[e][:, scc * TC : (scc + 1) * TC], box["pp"][:]
                        )
                    return [mk(k) for k in range(NK)] + [cp]

                def thunks_v(sb):
                    box = {}
                    def mk(k):
                        def f():
                            if k == 0:
                                box["pp"] = ppps.tile([SB, TC], F32,
                                                      tag="pp", name="pp")
                            nc.tensor.matmul(
                                box["pp"][:, 0:256],
                                hw[:, _hcol(k, sb * SB) :
                                   _hcol(k, sb * SB) + SB],
                                hw[:, _wcol(k, 512) : _wcol(k, 512) + 256],
                                start=(k == 0),
                                stop=(k == NKA - 1),
                            )
                            if with_bias and k == NK - 1:
                                nc.tensor.matmul(
                                    box["pp"][:, 0:256],
                                    hst9[:, sb * SB : (sb + 1) * SB],
                                    w9[:, 512:WK],
                                    start=False, stop=True,
                                )
                        return f
                    def cp():
                        vdst = bass_mod.AP(
                            tensor=vt[sb][:].tensor, offset=0,
                            ap=[[NHC * (HD + 1), SB], [HD + 1, NHC],
                                [1, HD]],
                        )
                        nc.vector.tensor_copy(vdst, box["pp"][:, 0:256])
                    return [mk(k) for k in range(NK)] + [cp]

                def alloc_pvh(pool="pv"):
                    """PV accumulator pair: (tile, col_base, row_stride).
                    pool="pp" borrows the projection pool (idle once the
                    last v thunk copied out) so the tail chunk's PV doesn't
                    serialize on pvps WARs."""
                    if pool == "pp":
                        return [(ppps.tile([SB, TC], F32, tag="pp",
                                           name=f"pvp{h2}"), 0, TC)
                                for h2 in range(2)]
                    return [(pvps.tile([SB, 260], F32, tag="pv",
                                       name=f"pv{h2}"), 0, 260)
                            for h2 in range(2)]

                def emit_pv_group(pair, pvh, tes, tb, tci, nsbs, ltb=None):
                    # one t-block accumulation group; groups sharing a PSUM
                    # bank must run strictly sequentially (a start=True marks
                    # the whole 2KB bank pending-zero, so an interleaved
                    # foreign start would turn accumulates into overwrites).
                    # ltb = column index inside the accumulator (defaults to
                    # tb; the tail chunk spreads tbs over separate banks).
                    last = _sb_last(tb, tci, nsbs)
                    if ltb is None:
                        ltb = tb
                    for h2 in range(2):
                        t, base, _ = pvh[h2]
                        for sb2 in range(last + 1):
                            teb = tes[sb2][:, h2 * TC + tb * SB :
                                           h2 * TC + (tb + 1) * SB]
                            nc.tensor.matmul(
                                t[:, base + 65 * ltb :
                                  base + 65 * ltb + HD + 1],
                                teb,
                                vt[sb2][:, (2 * pair + h2) * (HD + 1) :
                                        (2 * pair + h2 + 1) * (HD + 1)],
                                start=(sb2 == 0),
                                stop=(sb2 == last),
                                skip_group_check=True,
                            )

                def emit_norm(pair, tci, pvh, tb0, tb1, act_dma=False,
                              dma_eng=None, tb_off=0):
                    # per-partition Z -> reciprocal + free-broadcast mul;
                    # tb0/tb1 index columns inside the accumulator, tb_off
                    # shifts the DRAM destination t-block
                    ntb = tb1 - tb0
                    r = rpool.tile([SB, 8], F32, tag="r", name="r")
                    for h2 in range(2):
                        t, base, stride = pvh[h2]
                        zsrc = bass_mod.AP(
                            tensor=t[:].tensor,
                            offset=base + 65 * tb0 + HD,
                            ap=[[stride, SB], [65, ntb], [1, 1]],
                        )
                        nc.vector.reciprocal(
                            r[:, 4 * h2 : 4 * h2 + ntb], zsrc)
                    osb = opool.tile([SB, SB * ntb], F32, tag="osb",
                                     name="osb")
                    for h2 in range(2):
                        t, base, stride = pvh[h2]
                        odst = bass_mod.AP(
                            tensor=osb[:].tensor, offset=h2 * HD,
                            ap=[[SB * ntb, SB], [SB, ntb], [1, HD]],
                        )
                        psrc = bass_mod.AP(
                            tensor=t[:].tensor, offset=base + 65 * tb0,
                            ap=[[stride, SB], [65, ntb], [1, HD]],
                        )
                        rsrc = bass_mod.AP(
                            tensor=r[:].tensor, offset=4 * h2,
                            ap=[[8, SB], [1, ntb], [0, HD]],
                        )
                        nc.vector.tensor_mul(odst, psrc, rsrc)
                    skip512 = (tci == 1 and tb0 + tb_off == 0)
                    eng = dma_eng or (nc.scalar if act_dma else nc.sync)
                    if not skip512:
                        # single store covering all ntb t-blocks
                        dst = bass_mod.AP(
                            tensor=out_d.ap().tensor,
                            offset=(tci * TC + (tb0 + tb_off) * SB) * 256
                            + 128 * pair,
                            ap=[[256, SB], [SB * 256, ntb], [1, SB]],
                        )
                        src = bass_mod.AP(
                            tensor=osb[:].tensor, offset=0,
                            ap=[[SB * ntb, SB], [SB, ntb], [1, SB]],
                        )
                        eng.dma_start(out=dst, in_=src)
                        return
                    dst = bass_mod.AP(
                        tensor=out_d.ap().tensor,
                        offset=(tci * TC + (tb0 + tb_off) * SB + skip512)
                        * 256 + 128 * pair,
                        ap=[[256, SB - skip512], [SB * 256, 1], [1, SB]],
                    )
                    src = bass_mod.AP(
                        tensor=osb[:].tensor, offset=skip512 * SB * ntb,
                        ap=[[SB * ntb, SB - skip512], [SB, 1], [1, SB]],
                    )
                    eng.dma_start(out=dst, in_=src)
                    if ntb > 1:
                        dst2 = bass_mod.AP(
                            tensor=out_d.ap().tensor,
                            offset=(tci * TC + (tb0 + tb_off + 1) * SB)
                            * 256 + 128 * pair,
                            ap=[[256, SB], [SB * 256, ntb - 1], [1, SB]],
                        )
                        src2 = bass_mod.AP(
                            tensor=osb[:].tensor, offset=SB,
                            ap=[[SB * ntb, SB], [SB, ntb - 1], [1, SB]],
                        )
                        eng.dma_start(out=dst2, in_=src2)

                def emit_attn(pair, tci, feed=(), alt_pqk=False,
                              sb_order=None):
                    """QK + exp for every alive s-block; PV is deferred
                    (run emit_pv_chunk with the returned te list later)."""
                    sbs = _alive_sbs(tci)
                    n = len(sbs)
                    if sb_order is not None:
                        sbs = sb_order
                    feed = list(feed)
                    rate = -(-len(feed) // n) if feed else 0
                    tes = [None] * n
                    for i, sb in enumerate(sbs):
                        xlo = _x_lo(sb, tci)
                        if alt_pqk and i % 2 == 1:
                            pq2 = [ppps.tile([SB, TC], F32, tag="pp",
                                             name="pqh") for _ in range(2)]
                            halves = [pq2[h][:, xlo:TC] for h in range(2)]
                        else:
                            pqk = mmps.tile([SB, 2 * TC], F32, tag="mm",
                                            name="pqk")
                            halves = [pqk[:, h * TC + xlo : (h + 1) * TC]
                                      for h in range(2)]
                        for h2 in range(2):
                            nc.tensor.matmul(
                                halves[h2],
                                kt[pair][h2 * HD : (h2 + 1) * HD,
                                         sb * SB : (sb + 1) * SB],
                                qt[pair][h2 * HD : (h2 + 1) * HD,
                                         tci * TC + xlo : (tci + 1) * TC],
                                start=True,
                                stop=True,
                                tile_position=(h2 * HD, 0),
                            )
                        te = epool.tile([SB, 2 * TC], BF16, tag="te",
                                        name="te")
                        te3 = te[:].rearrange("p (b n) -> p b n", b=2)
                        if alt_pqk and i % 2 == 1:
                            for h2 in range(2):
                                nc.scalar.activation(
                                    te[:, h2 * TC + xlo : (h2 + 1) * TC],
                                    halves[h2],
                                    EXP,
                                    bias=attc[:, sb : sb + 1],
                                    scale=SCALE,
                                )
                        else:
                            pqk3 = pqk[:].rearrange("p (b n) -> p b n", b=2)
                            nc.scalar.activation(
                                te3[:, :, xlo:TC],
                                pqk3[:, :, xlo:TC],
                                EXP,
                                bias=attc[:, sb : sb + 1],
                                scale=SCALE,
                            )
                        for blk, mi, coff in _diag_actions(sb, tci):
                            x = blk * SB + coff
                            m_ap = bass_mod.AP(
                                tensor=masks[:].tensor,
                                offset=mi * SB + coff,
                                ap=[[2 * SB + 1, SB], [0, 2], [1, SB - coff]],
                            )
                            nc.vector.tensor_mul(
                                te3[:, :, x : blk * SB + SB],
                                te3[:, :, x : blk * SB + SB],
                                m_ap,
                            )
                        tes[sb] = te
                        # interleave projection / deferred-PV work for PE
                        for f in feed[i * rate : (i + 1) * rate]:
                            f()
                    for f in feed[n * rate :]:
                        f()
                    return tes

                def emit_pv_chunk(pair, tci, tes, tail_split=False,
                                  pool="pv"):
                    """Deferred PV+norm for a chunk whose QK ran earlier."""
                    n = len(_alive_sbs(tci))
                    pvh = alloc_pvh(pool)
                    for tb in range(4):
                        emit_pv_group(pair, pvh, tes, tb, tci, n)
                        if tail_split and tb < 3:
                            # per-tb norm+DMA so the HWDGE queue is clear
                            # when the final block's store issues
                            emit_norm(pair, tci, pvh, tb, tb + 1,
                                      act_dma=True)
                    if tail_split:
                        emit_norm(pair, tci, pvh, 3, 4, dma_eng=nc.sync)
                    else:
                        emit_norm(pair, tci, pvh, 0, 4)

                def emit_pv_tail(pair, tci, tes):
                    """Tail-chunk PV: t-blocks spread over separate banks so
                    each block's norm+store flows without serializing the
                    next block's start=True (whole-bank pending-zero WAR)."""
                    n = len(_alive_sbs(tci))
                    ppA = ppps.tile([SB, TC], F32, tag="pp", name="pvtA")
                    ppB = ppps.tile([SB, TC], F32, tag="pp", name="pvtB")
                    pvC = pvps.tile([SB, 260], F32, tag="pv", name="pvtC")
                    phA = [(ppA, 0, TC), (ppA, 256, TC)]
                    phB = [(ppB, 0, TC), (ppB, 256, TC)]
                    phC = [(pvC, 0, 260), (pvC, 130, 260)]
                    emit_pv_group(pair, phA, tes, 0, tci, n, ltb=0)
                    emit_pv_group(pair, phA, tes, 1, tci, n, ltb=1)
                    emit_norm(pair, tci, phA, 0, 2, act_dma=True)
                    emit_pv_group(pair, phB, tes, 2, tci, n, ltb=0)
                    emit_norm(pair, tci, phB, 0, 1, act_dma=True, tb_off=2)
                    emit_pv_group(pair, phC, tes, 3, tci, n, ltb=0)
                    emit_norm(pair, tci, phC, 0, 1, dma_eng=nc.sync,
                              tb_off=3)

                def emit_special_probs():
                    """t=512 scores, column-major: psp[s, 4g+sb]; one exp."""
                    psp = ppps.tile([SB, TC], F32, tag="pp", name="psp")
                    for g in range(NHC):
                        pair, h2 = g // 2, g % 2
                        qcol = qt[pair][h2 * HD : (h2 + 1) * HD, 512:513]
                        for sb in range(4):
                            nc.tensor.matmul(
                                psp[:, 4 * g + sb : 4 * g + sb + 1],
                                kt[pair][h2 * HD : (h2 + 1) * HD,
                                         sb * SB : (sb + 1) * SB],
                                qcol,
                                start=True, stop=True,
                                skip_group_check=True,
                                tile_position=(h2 * HD, 0),
                            )
                        # j=512 tail term (partition 0)
                        nc.tensor.matmul(
                            psp[0:1, 16 + g : 17 + g],
                            kt[pair][h2 * HD : (h2 + 1) * HD, 512:513],
                            qcol,
                            start=True, stop=True,
                            skip_group_check=True,
                            tile_position=(h2 * HD, 0),
                        )
                    if with_attc:
                        am = bass_mod.AP(
                            tensor=attcs[:].tensor, offset=0,
                            ap=[[NSB, SB], [0, NHC], [1, 4]],
                        )
                        nc.vector.tensor_add(psp[:, 0:16], psp[:, 0:16], am)
                        am5 = bass_mod.AP(
                            tensor=attr_t[:].tensor, offset=512,
                            ap=[[0, 1], [0, NHC]],
                        )
                        nc.vector.tensor_add(
                            psp[0:1, 16:20], psp[0:1, 16:20], am5
                        )
                    nc.scalar.activation(
                        erT[:, 0:20], psp[:, 0:20], EXP, scale=SCALE
                    )

                def emit_special_pv():
                    """Row-major PV from the column-major probs in erT."""
                    pvs = pvps.tile([SB, 260], F32, tag="pv", name="pvs")
                    for g in range(NHC):
                        for sb4 in range(4):
                            nc.tensor.matmul(
                                pvs[0:1, g * (HD + 1) :
                                    (g + 1) * (HD + 1)],
                                erT[:, 4 * g + sb4 : 4 * g + sb4 + 1],
                                vt[sb4][:, g * (HD + 1) :
                                        (g + 1) * (HD + 1)],
                                start=(sb4 == 0),
                                stop=False,
                                skip_group_check=True,
                            )
                        nc.tensor.matmul(
                            pvs[0:1, g * (HD + 1) : (g + 1) * (HD + 1)],
                            erT[0:1, 16 + g : 17 + g],
                            vt[4][0:1, g * (HD + 1) : (g + 1) * (HD + 1)],
                            start=False, stop=True,
                            skip_group_check=True,
                        )
                    zsrc = bass_mod.AP(
                        tensor=pvs[:].tensor, offset=HD,
                        ap=[[260, 1], [HD + 1, NHC], [1, 1]],
                    )
                    nc.vector.reciprocal(rz4[0:1, 0:NHC], zsrc)
                    for g in range(NHC):
                        nc.vector.tensor_scalar_mul(
                            svn[0:1, g * HD : (g + 1) * HD],
                            pvs[0:1, g * (HD + 1) :
                                g * (HD + 1) + HD],
                            rz4[0:1, g : g + 1],
                        )
                    nc.sync.dma_start(out=out_d[512:513, :], in_=svn[:])

                # startup: q00/k00 only, k-major, riding the DMA pipeline.
                # v projections move to feed (their only consumers, the PVs,
                # are deferred) so the first exp lands ~5us earlier and ACT
                # saturates sooner.
                pre_qk = mmps.tile([SB, 2 * TC], F32, tag="mm", name="preqk")
                # k0 in t-halves (t 0:256 arrives in the first DMA piece)
                for half in range(2):
                    for col0 in (0, TC):
                        nc.tensor.matmul(
                            pre_qk[:, col0 + half * 256 :
                                   col0 + (half + 1) * 256],
                            hw[:, _wcol(0, col0 // 4) :
                               _wcol(0, col0 // 4) + SB],
                            hw[:, _hcol(0, half * 256) :
                               _hcol(0, half * 256) + 256],
                            start=(half == 0), stop=False,
                        )
                for k in range(1, NK):
                    nc.tensor.matmul(
                        pre_qk[:, 0:TC],
                        hw[:, _wcol(k, 0) : _wcol(k, 0) + SB],
                        hw[:, _hcol(k, 0) : _hcol(k, 0) + TC],
                        start=False, stop=(k == NKA - 1),
                    )
                    nc.tensor.matmul(
                        pre_qk[:, TC : 2 * TC],
                        hw[:, _wcol(k, SB) : _wcol(k, SB) + SB],
                        hw[:, _hcol(k, 0) : _hcol(k, 0) + TC],
                        start=False, stop=(k == NKA - 1),
                    )
                if with_bias:
                    nc.tensor.matmul(
                        pre_qk[:, 0:TC], w9[:, 0:SB], hst9[:, 0:TC],
                        start=False, stop=True,
                    )
                    nc.tensor.matmul(
                        pre_qk[:, TC : 2 * TC], w9[:, SB : 2 * SB],
                        hst9[:, 0:TC], start=False, stop=True,
                    )
                # piecewise copies, descending halves, so L00's QK
                # (descending sb order) starts as soon as its slice lands;
                # qt halves go via the idle GPSIMD engine in parallel
                for piece in (1, 0):
                    nc.vector.tensor_copy(
                        kt[0][:, piece * 256 : (piece + 1) * 256],
                        pre_qk[:, TC + piece * 256 : TC + (piece + 1) * 256])
                    nc.vector.tensor_copy(
                        qt[0][:, piece * 256 : (piece + 1) * 256],
                        pre_qk[:, piece * 256 : (piece + 1) * 256])

                # v7 schedule: QK of every chunk as early as deps allow so
                # ACT (exp) saturates from ~10us on; all PV work floats to
                # wherever PE has slack vs ACT, finishing with a pure-PE
                # endgame that runs while ACT drains the last exps.
                q01 = thunks_q(0, 1)
                v0 = thunks_v(0)
                te00 = emit_attn(0, 0, feed=v0[:5] + q01[:4]
                                 + v0[5:] + q01[4:],
                                 sb_order=[3, 2, 1, 0])
                te01 = emit_attn(0, 1, feed=thunks_k(0, 1) + thunks_q(0, 2))
                te02 = emit_attn(0, 2, feed=thunks_v(1) + thunks_q(1, 0)
                                 + thunks_k(1, 0))
                te10 = emit_attn(1, 0, feed=thunks_q(1, 1) + thunks_k(1, 1))
                te11 = emit_attn(1, 1, feed=thunks_q(0, 3) + thunks_k(0, 2))
                emit_special_probs()
                te03 = emit_attn(
                    0, 3,
                    feed=thunks_q(1, 2) + thunks_v(2) + thunks_v(3)
                    + thunks_v(4) + thunks_v(5) + thunks_v(6)
                    + thunks_v(7))
                q13 = thunks_q(1, 3)
                k12 = thunks_k(1, 2)
                te12 = emit_attn(
                    1, 2,
                    feed=q13[:5] + [lambda: emit_pv_chunk(0, 0, te00)]
                    + q13[5:] + [lambda: emit_pv_chunk(0, 1, te01)]
                    + k12[:5] + [lambda: emit_pv_chunk(1, 0, te10)]
                    + k12[5:] + [lambda: emit_pv_chunk(1, 1, te11)])
                te13 = emit_attn(
                    1, 3,
                    feed=thunks_v(8)
                    + [lambda: emit_pv_chunk(0, 2, te02)]
                    + thunks_v(9)
                    + [lambda: emit_pv_chunk(1, 2, te12)]
                    + thunks_v(10) + [emit_special_pv]
                    + thunks_v(11)
                    + [lambda: emit_pv_chunk(0, 3, te03)])
                # endgame: only the last chunk's PV remains after the final
                # exps; its accumulators borrow the idle projection pool so
                # nothing serializes on pvps WARs
                emit_pv_tail(1, 3, te13)

    nc.compile()
    return nc


def _host_prep(inputs, with_bias, with_attc):
    import ml_dtypes

    bf = ml_dtypes.bfloat16
    hs = np.asarray(inputs["hidden_states"], dtype=np.float32)
    am = np.asarray(inputs["attention_mask"], dtype=np.float32)
    Wq = np.asarray(inputs["Wq"], dtype=np.float32)
    bq = np.asarray(inputs["bq"], dtype=np.float32)
    Wk = np.asarray(inputs["Wk"], dtype=np.float32)
    bk = np.asarray(inputs["bk"], dtype=np.float32)
    Wv = np.asarray(inputs["Wv"], dtype=np.float32)
    bv = np.asarray(inputs["bv"], dtype=np.float32)

    p = np.arange(SB)[:, None]
    x = np.arange(SB)[None, :]
    m0 = (p <= x).astype(np.float32)
    m1 = (p <= x - 1).astype(np.float32)
    ones = np.ones((SB, 1), dtype=np.float32)
    masks = np.concatenate([m0, m1, ones], axis=1).astype(bf)

    in_maps = []
    for c in range(8):
        b, g = c // 4, c % 4
        # hst [128, k, 2048]: hst[p, k, t] = hs[b, t, 128k+p]
        hsT = hs[b].T  # [1024, 2048]
        hstp = hsT.reshape(NK, SB, S).transpose(1, 0, 2)
        # w [128, k, 768]: [q0|k0|q1|k1|v], w[p,k,256e+j]=W[256g+128e+j,128k+p]
        w = np.zeros((SB, NK, WK), dtype=np.float32)
        Wq_sl = Wq[256 * g : 256 * (g + 1), :]  # [256, 1024]
        Wk_sl = Wk[256 * g : 256 * (g + 1), :]
        Wv_sl = Wv[256 * g : 256 * (g + 1), :]
        for k in range(NK):
            cols = slice(SB * k, SB * (k + 1))
            w[:, k, 0:128] = Wq_sl[0:128, cols].T
            w[:, k, 128:256] = Wk_sl[0:128, cols].T
            w[:, k, 256:384] = Wq_sl[128:256, cols].T
            w[:, k, 384:512] = Wk_sl[128:256, cols].T
            w[:, k, 512:768] = Wv_sl[:, cols].T
        # interleave into need-ordered sections (see _wcol/_hcol)
        hwm = np.zeros((SB, NK, SEC), dtype=np.float32)
        hwm[:, :, 0:256] = w[:, :, 0:256]
        hwm[:, :, 256:768] = hstp[:, :, 0:512]
        hwm[:, :, 768:1024] = w[:, :, 512:768]
        hwm[:, :, 1024:1280] = w[:, :, 256:512]
        hwm[:, :, 1280:1792] = hstp[:, :, 512:1024]
        hwm[:, :, 1792:2304] = hstp[:, :, 1024:1536]
        hwm[:, :, 2304:2816] = hstp[:, :, 1536:2048]
        amv = am[b, 0, 0, :].astype(np.float32)
        attc = np.ascontiguousarray(amv.reshape(NSB, SB).T)
        m = {
            "hw": hwm.reshape(SB, NK * SEC).astype(bf),
            "attc": attc,
            "masks": masks.copy(),
        }
        if with_attc:
            m["attr"] = (amv[:513] / SCALE).reshape(1, 513).copy()
            m["attcs"] = (attc / SCALE).copy()
        if with_bias:
            bsl = np.zeros((WK,), dtype=np.float32)
            bsl[0:128] = bq[256 * g : 256 * g + 128]
            bsl[128:256] = bk[256 * g : 256 * g + 128]
            bsl[256:384] = bq[256 * g + 128 : 256 * g + 256]
            bsl[384:512] = bk[256 * g + 128 : 256 * g + 256]
            bsl[512:768] = bv[256 * g : 256 * (g + 1)]
            m["hst9"] = np.ones((1, S), dtype=np.float32).astype(bf)
            m["w9"] = bsl.reshape(1, WK).astype(bf)
        in_maps.append(m)
    return in_maps


LAST_EXEC_NS = None


def kernel(**inputs):
    import os

    from concourse.bass_utils import run_bass_kernel_spmd

    global LAST_EXEC_NS
    with_bias = bool(
        np.any(np.asarray(inputs["bq"]))
        or np.any(np.asarray(inputs["bk"]))
        or np.any(np.asarray(inputs["bv"]))
    )
    with_attc = bool(np.any(np.asarray(inputs["attention_mask"])))
    key = f"nc{int(with_bias)}{int(with_attc)}"
    if key not in _CACHE:
        _CACHE[key] = _build_program(with_bias=with_bias,
                                     with_attc=with_attc)
    nc = _CACHE[key]
    in_maps = _host_prep(inputs, with_bias, with_attc)
    trace = bool(os.environ.get("BASS_KERNEL_TRACE"))
    res = run_bass_kernel_spmd(nc, in_maps, list(range(8)), trace=trace)
    LAST_EXEC_NS = res.exec_time_ns
    out = np.empty((B, S, H), dtype=np.float32)
    for c in range(8):
        b, g = c // 4, c % 4
        out[b, :, 256 * g : 256 * (g + 1)] = res.results[c]["out"]
    return out


# revision 64
# speedup vs baseline: 1.0038x; 1.0038x over previous
"""Trainium2 Bass kernel for nn_CausalSelfAttention_2860448219236 (v5).

Reference semantics (B=2, S=2048, H=1024, NH=16, HD=64, WINDOW=512, NEG=-1e4):
  q/k/v = heads(hs @ W{q,k,v}.T + b)
  mask  = causal(j>i: NEG) + window(j >= i-512: NEG) + attention_mask
  out   = softmax(q k^T/8 + mask) v

Because NEG=-1e4 and softmax subtracts the row max, the f32 result equals a
*binary*-masked softmax over the allowed set
  A(i) = {j <= i}        for i <= 512  (whole row carries the same -1e4)
       = {j <= i-513}    for i >= 513  (recent-window entries underflow to 0)

Sharding: core c = (batch b = c//4) x (head group g = c%4, heads 4g..4g+3).
Fully data-parallel SPMD - one program, per-core input slices, no collectives.

v5 design (cost model: matmul time = out-free-size rows; exp on ACT is the
second-busiest engine, ~49us vs PE ~63us):
  - single interleaved DRAM input `hw`: per k-section the columns are ordered
    by when they are first needed [wqk-e0 | hst-c0 | wqk-e1 | hst-c1 | wv |
    hst-c2 | hst-c3], so startup needs only 4 merged DMAs before the first
    matmul and 13 input DMAs total (each DMA costs ~625ns of serialized HWDGE
    hold + ~2.2us fixed latency).
  - QK of every chunk emitted as early as deps allow so ACT saturates early;
    ALL PV work is deferred and floated to wherever PE has slack vs ACT
    (emit_pv_chunk as feed thunks), ending with a pure-PE endgame that runs
    while ACT drains the last chunk's exps. te tiles stay alive until their
    PV (epool bufs=56 covers every allocation - no recycling).
  - t=512 special pass in column orientation: 20 free-size-1 score matmuls
    into one PSUM bank (sequential start/stop groups per column - data of
    completed columns survives later bank pending-zero marks), ONE 20-wide
    exp, then row-major PV off the column probs. ~0.2us PE + ~0.2us ACT and
    zero DRAM round-trips.
  - scoresT[s,t] grid: QK packs 2 heads via row tile_position; exp on ACT
    with per-partition attn-mask bias; 0/1 diagonal masks on DVE.
  - PV transposed: out[t,d] = te(block)^T @ v -> 65-row matmuls; Z via the
    ones column in v. Normalization per-partition: reciprocal of Z cols +
    broadcast mul; output DRAM layout [t, d].
  - t=512 (the one column whose window boundary is not block-aligned) is
    recomputed exactly by the special pass; the main grid leaves tci=1/tb=4
    col 0 unmasked so Z!=0 and skips storing that row.
"""

import numpy as np

S = 2048
H = 1024
B = 2
NH = 16
HD = 64
SCALE = 0.125
SB = 128          # s block
TC = 512          # t chunk
NTC = S // TC     # 4
NSB = S // SB     # 16
NHC = 4           # heads per core
KTS = S - 512     # kT/v s-extent needed by the main grid (1536)
NVT = KTS // SB   # 12 v tiles
NK = H // SB      # 8 k-tiles
WK = 768          # per-k-tile w cols: q0|k0|q1|k1|v(256)
SEC = WK + S      # interleaved per-k section: 2816 cols
# region offsets inside a k-section: [q0k0 | hst-c0 | wv | q1k1 | hst-c1 |
# hst-c2 | hst-c3] so the pre phase (q00/k00) needs only cols 0:768
_WOFS = (0, 1024, 768)        # wqk-e0, wqk-e1, wv
_HOFS = (256, 1280, 1792, 2304)  # hst chunks 0..3

_CACHE = {}


def _wcol(k, c):
    """w column c (0..WK) of k-tile k -> interleaved hw column."""
    r, o = divmod(c, 256)
    return k * SEC + _WOFS[r] + o


def _hcol(k, t):
    """hst column t (0..S) of k-tile k -> interleaved hw column."""
    cc, o = divmod(t, TC)
    return k * SEC + _HOFS[cc] + o


def _alive_sbs(tci):
    if tci == 0:
        return list(range(4))
    return list(range(min(4 * tci, NVT)))


def _x_lo(sb, tci):
    first_tb = sb if tci == 0 else sb + 4
    return max(0, SB * (first_tb - 4 * tci))


def _sb_last(tb, tci, nsbs):
    # last sb contributing to t-block tb of chunk tci
    if tci == 0:
        return tb
    return min(nsbs - 1, 4 * tci - 4 + tb)


def _diag_actions(sb, tci):
    """[(block_in_chunk, mask_idx, col_off)]; mask 0 = p<=x, 1 = p<=x-1."""
    acts = []
    for tb in range(4 * tci, 4 * tci + 4):
        if tb <= 3 and tb == sb:
            acts.append((tb - 4 * tci, 0, 0))
        if tb >= 4 and tb - 4 == sb:
            # at t0=512 (tci==1, block 0) leave col 0 unmasked: that column
            # (t=512) is recomputed exactly by the special pass, and masking
            # it fully would make Z=0 -> div noise.
            col_off = 1 if (tci == 1 and tb == 4) else 0
            acts.append((tb - 4 * tci, 1, col_off))
    return acts


def _build_program(with_bias=False, with_attc=False):
    import concourse.bass as bass_mod
    import concourse.bacc as bacc
    import concourse.mybir as mybir
    from concourse.tile import TileContext

    F32 = mybir.dt.float32
    BF16 = mybir.dt.bfloat16
    EXP = mybir.ActivationFunctionType.Exp

    nc = bacc.Bacc("TRN2", target_bir_lowering=False, debug=False)

    hw_d = nc.dram_tensor("hw", [SB, NK * SEC], BF16, kind="ExternalInput")
    attc_d = nc.dram_tensor("attc", [SB, NSB], F32, kind="ExternalInput")
    masks_d = nc.dram_tensor("masks", [SB, 2 * SB + 1], BF16,
                             kind="ExternalInput")
    if with_attc:
        attr_d = nc.dram_tensor("attr", [1, 513], F32, kind="ExternalInput")
        attcs_d = nc.dram_tensor("attcs", [SB, NSB], F32,
                                 kind="ExternalInput")
    if with_bias:
        hst9_d = nc.dram_tensor("hst9", [1, S], BF16, kind="ExternalInput")
        w9_d = nc.dram_tensor("w9", [1, WK], BF16, kind="ExternalInput")
    out_d = nc.dram_tensor("out", [S, NHC * HD], BF16,
                       kind="ExternalOutput")

    NKA = NK + (1 if with_bias else 0)

    with TileContext(nc) as tc:
        with tc.tile_pool(name="stat", bufs=1) as stat:
            hw = stat.tile([SB, NK * SEC], BF16, tag="hw", name="hw")
            qt = [stat.tile([SB, S], BF16, tag=f"qt{e}", name=f"qt{e}")
                  for e in range(2)]
            kt = [stat.tile([SB, KTS], BF16, tag=f"kt{e}", name=f"kt{e}")
                  for e in range(2)]
            vt = [stat.tile([SB, NHC * (HD + 1)], BF16, tag=f"vt{i}",
                            name=f"vt{i}")
                  for i in range(NVT)]
            attc = stat.tile([SB, NSB], F32, tag="attc", name="attc")
            masks = stat.tile([SB, 2 * SB + 1], BF16, tag="masks",
                              name="masks")
            # special pass (t=512): erT[s, 4g+sb] = probs column-major;
            # cols 16..19 (partition 0 only) hold the j=512 tail per g.
            erT = stat.tile([SB, 20], BF16, tag="erT", name="erT")
            rz4 = stat.tile([1, NHC], F32, tag="rz4", name="rz4")
            svn = stat.tile([1, NHC * HD], BF16, tag="svn", name="svn")
            if with_attc:
                attr_t = stat.tile([1, 513], F32, tag="attr", name="attr")
                attcs = stat.tile([SB, NSB], F32, tag="attcs", name="attcs")
            if with_bias:
                hst9 = stat.tile([1, S], BF16, tag="hst9", name="hst9")
                w9 = stat.tile([1, WK], BF16, tag="w9", name="w9")

            # warm the ACT exp table during the DMA-bound startup: the
            # framework inserts the 1.3us LoadActFuncSet before the first
            # activation, so make that a dummy one at t~0


            # warm the ACT exp table during the DMA-bound startup: the
            # framework inserts the 1.3us LoadActFuncSet before the first
            # activation, so make that a dummy one at t~0 (erT cols are
            # overwritten by the real special-pass exp later)
            nc.vector.memset(erT[0:1, 0:4], 0.0)
            nc.scalar.activation(erT[0:1, 4:8], erT[0:1, 0:4], EXP)

            for i in range(NVT):
                ocols = bass_mod.AP(
                    tensor=vt[i][:].tensor, offset=HD,
                    ap=[[NHC * (HD + 1), SB], [HD + 1, NHC], [1, 1]],
                )
                nc.vector.memset(ocols, 1.0)

            # --- merged, need-ordered input DMA (SP queue) ----------------
            def dma_cols(c0, c1, k0, k1):
                src = bass_mod.AP(
                    tensor=hw_d.ap().tensor, offset=k0 * SEC + c0,
                    ap=[[NK * SEC, SB], [SEC, k1 - k0], [1, c1 - c0]],
                )
                dst = bass_mod.AP(
                    tensor=hw[:].tensor, offset=k0 * SEC + c0,
                    ap=[[NK * SEC, SB], [SEC, k1 - k0], [1, c1 - c0]],
                )
                nc.sync.dma_start(out=dst, in_=src)

            # first piece split so the very first matmul (k0, t 0:256) can
            # start ~0.6us sooner
            dma_cols(0, 512, 0, 1)                 # k0: q0k0 + hst t0:256
            dma_cols(512, 768, 0, 1)               # k0: hst t256:512
            for kk in range(1, NK):
                dma_cols(0, 768, kk, kk + 1)       # q0k0 + hst-c0
            dma_cols(768, 1280, 0, 4)              # wv + wqk-e1
            dma_cols(768, 1280, 4, NK)
            nc.sync.dma_start(out=masks[:], in_=masks_d[:])
            nc.sync.dma_start(out=attc[:], in_=attc_d[:])
            if with_attc:
                nc.sync.dma_start(out=attr_t[:], in_=attr_d[:])
                nc.sync.dma_start(out=attcs[:], in_=attcs_d[:])
            if with_bias:
                nc.sync.dma_start(out=hst9[:], in_=hst9_d[:])
                nc.sync.dma_start(out=w9[:], in_=w9_d[:])
            dma_cols(1280, 1792, 0, 4)             # hst-c1
            dma_cols(1280, 1792, 4, NK)
            dma_cols(1792, 2304, 0, 4)             # hst-c2
            dma_cols(1792, 2304, 4, NK)
            dma_cols(2304, 2816, 0, 4)             # hst-c3
            dma_cols(2304, 2816, 4, NK)

            with (
                tc.tile_pool(name="mmps", bufs=2, space="PSUM") as mmps,
                tc.tile_pool(name="ppps", bufs=2, space="PSUM") as ppps,
                tc.tile_pool(name="pvps", bufs=2, space="PSUM") as pvps,
                tc.tile_pool(name="epool", bufs=56) as epool,
                tc.tile_pool(name="rpool", bufs=6) as rpool,
                tc.tile_pool(name="opool", bufs=6) as opool,
            ):

                def thunks_q(e, tcc):
                    box = {}
                    def mk(k):
                        def f():
                            if k == 0:
                                box["pp"] = ppps.tile([SB, TC], F32,
                                                      tag="pp", name="pp")
                            nc.tensor.matmul(
                                box["pp"][:],
                                hw[:, _wcol(k, 256 * e) :
                                   _wcol(k, 256 * e) + SB],
                                hw[:, _hcol(k, tcc * TC) :
                                   _hcol(k, tcc * TC) + TC],
                                start=(k == 0),
                                stop=(k == NKA - 1),
                            )
                            if with_bias and k == NK - 1:
                                nc.tensor.matmul(
                                    box["pp"][:],
                                    w9[:, 256 * e : 256 * e + SB],
                                    hst9[:, tcc * TC : (tcc + 1) * TC],
                                    start=False, stop=True,
                                )
                        return f
                    def cp():
                        nc.vector.tensor_copy(
                            qt[e][:, tcc * TC : (tcc + 1) * TC], box["pp"][:]
                        )
                    return [mk(k) for k in range(NK)] + [cp]

                def thunks_k(e, scc):
                    box = {}
                    off0 = 256 * e + SB
                    def mk(k):
                        def f():
                            if k == 0:
                                box["pp"] = ppps.tile([SB, TC], F32,
                                                      tag="pp", name="pp")
                            nc.tensor.matmul(
                                box["pp"][:],
                                hw[:, _wcol(k, off0) : _wcol(k, off0) + SB],
                                hw[:, _hcol(k, scc * TC) :
                                   _hcol(k, scc * TC) + TC],
                                start=(k == 0),
                                stop=(k == NKA - 1),
                            )
                            if with_bias and k == NK - 1:
                                nc.tensor.matmul(
                                    box["pp"][:],
                                    w9[:, off0 : off0 + SB],
                                    hst9[:, scc * TC : (scc + 1) * TC],
                                    start=False, stop=True,
                                )
                        return f
                    def cp():
                        nc.vector.tensor_copy(
                            kt[e][:, scc * TC : (scc + 1) * TC], box["pp"][:]
                        )
                    return [mk(k) for k in range(NK)] + [cp]

                def thunks_v(sb):
                    box = {}
                    def mk(k):
                        def f():
                            if k == 0:
                                box["pp"] = ppps.tile([SB, TC], F32,
                                                      tag="pp", name="pp")
                            nc.tensor.matmul(
                                box["pp"][:, 0:256],
                                hw[:, _hcol(k, sb * SB) :
                                   _hcol(k, sb * SB) + SB],
                                hw[:, _wcol(k, 512) : _wcol(k, 512) + 256],
                                start=(k == 0),
                                stop=(k == NKA - 1),
                            )
                            if with_bias and k == NK - 1:
                                nc.tensor.matmul(
                                    box["pp"][:, 0:256],
                                    hst9[:, sb * SB : (sb + 1) * SB],
                                    w9[:, 512:WK],
                                    start=False, stop=True,
                                )
                        return f
                    def cp():
                        vdst = bass_mod.AP(
                            tensor=vt[sb][:].tensor, offset=0,
                            ap=[[NHC * (HD + 1), SB], [HD + 1, NHC],
                                [1, HD]],
                        )
                        nc.vector.tensor_copy(vdst, box["pp"][:, 0:256])
                    return [mk(k) for k in range(NK)] + [cp]

                def alloc_pvh(pool="pv"):
                    """PV accumulator pair: (tile, col_base, row_stride).
                    pool="pp" borrows the projection pool (idle once the
                    last v thunk copied out) so the tail chunk's PV doesn't
                    serialize on pvps WARs."""
                    if pool == "pp":
                        return [(ppps.tile([SB, TC], F32, tag="pp",
                                           name=f"pvp{h2}"), 0, TC)
                                for h2 in range(2)]
                    return [(pvps.tile([SB, 260], F32, tag="pv",
                                       name=f"pv{h2}"), 0, 260)
                            for h2 in range(2)]

                def emit_pv_group(pair, pvh, tes, tb, tci, nsbs, ltb=None):
                    # one t-block accumulation group; groups sharing a PSUM
                    # bank must run strictly sequentially (a start=True marks
                    # the whole 2KB bank pending-zero, so an interleaved
                    # foreign start would turn accumulates into overwrites).
                    # ltb = column index inside the accumulator (defaults to
                    # tb; the tail chunk spreads tbs over separate banks).
                    last = _sb_last(tb, tci, nsbs)
                    if ltb is None:
                        ltb = tb
                    for h2 in range(2):
                        t, base, _ = pvh[h2]
                        for sb2 in range(last + 1):
                            teb = tes[sb2][:, h2 * TC + tb * SB :
                                           h2 * TC + (tb + 1) * SB]
                            nc.tensor.matmul(
                                t[:, base + 65 * ltb :
                                  base + 65 * ltb + HD + 1],
                                teb,
                                vt[sb2][:, (2 * pair + h2) * (HD + 1) :
                                        (2 * pair + h2 + 1) * (HD + 1)],
                                start=(sb2 == 0),
                                stop=(sb2 == last),
                                skip_group_check=True,
                            )

                def emit_norm(pair, tci, pvh, tb0, tb1, act_dma=False,
                              dma_eng=None, tb_off=0):
                    # per-partition Z -> reciprocal + free-broadcast mul;
                    # tb0/tb1 index columns inside the accumulator, tb_off
                    # shifts the DRAM destination t-block
                    ntb = tb1 - tb0
                    r = rpool.tile([SB, 8], F32, tag="r", name="r")
                    for h2 in range(2):
                        t, base, stride = pvh[h2]
                        zsrc = bass_mod.AP(
                            tensor=t[:].tensor,
                            offset=base + 65 * tb0 + HD,
                            ap=[[stride, SB], [65, ntb], [1, 1]],
                        )
                        nc.vector.reciprocal(
                            r[:, 4 * h2 : 4 * h2 + ntb], zsrc)
                    osb = opool.tile([SB, SB * ntb], BF16, tag="osb",
                                     name="osb")
                    for h2 in range(2):
                        t, base, stride = pvh[h2]
                        odst = bass_mod.AP(
                            tensor=osb[:].tensor, offset=h2 * HD,
                            ap=[[SB * ntb, SB], [SB, ntb], [1, HD]],
                        )
                        psrc = bass_mod.AP(
                            tensor=t[:].tensor, offset=base + 65 * tb0,
                            ap=[[stride, SB], [65, ntb], [1, HD]],
                        )
                        rsrc = bass_mod.AP(
                            tensor=r[:].tensor, offset=4 * h2,
                            ap=[[8, SB], [1, ntb], [0, HD]],
                        )
                        nc.vector.tensor_mul(odst, psrc, rsrc)
                    skip512 = (tci == 1 and tb0 + tb_off == 0)
                    eng = dma_eng or (nc.scalar if act_dma else nc.sync)
                    if not skip512:
                        # single store covering all ntb t-blocks
                        dst = bass_mod.AP(
                            tensor=out_d.ap().tensor,
                            offset=(tci * TC + (tb0 + tb_off) * SB) * 256
                            + 128 * pair,
                            ap=[[256, SB], [SB * 256, ntb], [1, SB]],
                        )
                        src = bass_mod.AP(
                            tensor=osb[:].tensor, offset=0,
                            ap=[[SB * ntb, SB], [SB, ntb], [1, SB]],
                        )
                        eng.dma_start(out=dst, in_=src)
                        return
                    dst = bass_mod.AP(
                        tensor=out_d.ap().tensor,
                        offset=(tci * TC + (tb0 + tb_off) * SB + skip512)
                        * 256 + 128 * pair,
                        ap=[[256, SB - skip512], [SB * 256, 1], [1, SB]],
                    )
                    src = bass_mod.AP(
                        tensor=osb[:].tensor, offset=skip512 * SB * ntb,
                        ap=[[SB * ntb, SB - skip512], [SB, 1], [1, SB]],
                    )
                    eng.dma_start(out=dst, in_=src)
                    if ntb > 1:
                        dst2 = bass_mod.AP(
                            tensor=out_d.ap().tensor,
                            offset=(tci * TC + (tb0 + tb_off + 1) * SB)
                            * 256 + 128 * pair,
                            ap=[[256, SB], [SB * 256, ntb - 1], [1, SB]],
                        )
                        src2 = bass_mod.AP(
                            tensor=osb[:].tensor, offset=SB,
                            ap=[[SB * ntb, SB], [SB, ntb - 1], [1, SB]],
                        )
                        eng.dma_start(out=dst2, in_=src2)

                def emit_attn(pair, tci, feed=(), alt_pqk=False,
                              sb_order=None):
                    """QK + exp for every alive s-block; PV is deferred
                    (run emit_pv_chunk with the returned te list later)."""
                    sbs = _alive_sbs(tci)
                    n = len(sbs)
                    if sb_order is not None:
                        sbs = sb_order
                    feed = list(feed)
                    rate = -(-len(feed) // n) if feed else 0
                    tes = [None] * n
                    for i, sb in enumerate(sbs):
                        xlo = _x_lo(sb, tci)
                        if alt_pqk and i % 2 == 1:
                            pq2 = [ppps.tile([SB, TC], F32, tag="pp",
                                             name="pqh") for _ in range(2)]
                            halves = [pq2[h][:, xlo:TC] for h in range(2)]
                        else:
                            pqk = mmps.tile([SB, 2 * TC], F32, tag="mm",
                                            name="pqk")
                            halves = [pqk[:, h * TC + xlo : (h + 1) * TC]
                                      for h in range(2)]
                        for h2 in range(2):
                            nc.tensor.matmul(
                                halves[h2],
                                kt[pair][h2 * HD : (h2 + 1) * HD,
                                         sb * SB : (sb + 1) * SB],
                                qt[pair][h2 * HD : (h2 + 1) * HD,
                                         tci * TC + xlo : (tci + 1) * TC],
                                start=True,
                                stop=True,
                                tile_position=(h2 * HD, 0),
                            )
                        te = epool.tile([SB, 2 * TC], BF16, tag="te",
                                        name="te")
                        te3 = te[:].rearrange("p (b n) -> p b n", b=2)
                        if alt_pqk and i % 2 == 1:
                            for h2 in range(2):
                                nc.scalar.activation(
                                    te[:, h2 * TC + xlo : (h2 + 1) * TC],
                                    halves[h2],
                                    EXP,
                                    bias=attc[:, sb : sb + 1],
                                    scale=SCALE,
                                )
                        else:
                            pqk3 = pqk[:].rearrange("p (b n) -> p b n", b=2)
                            nc.scalar.activation(
                                te3[:, :, xlo:TC],
                                pqk3[:, :, xlo:TC],
                                EXP,
                                bias=attc[:, sb : sb + 1],
                                scale=SCALE,
                            )
                        for blk, mi, coff in _diag_actions(sb, tci):
                            x = blk * SB + coff
                            m_ap = bass_mod.AP(
                                tensor=masks[:].tensor,
                                offset=mi * SB + coff,
                                ap=[[2 * SB + 1, SB], [0, 2], [1, SB - coff]],
                            )
                            nc.vector.tensor_mul(
                                te3[:, :, x : blk * SB + SB],
                                te3[:, :, x : blk * SB + SB],
                                m_ap,
                            )
                        tes[sb] = te
                        # interleave projection / deferred-PV work for PE
                        for f in feed[i * rate : (i + 1) * rate]:
                            f()
                    for f in feed[n * rate :]:
                        f()
                    return tes

                def emit_pv_chunk(pair, tci, tes, tail_split=False,
                                  pool="pv"):
                    """Deferred PV+norm for a chunk whose QK ran earlier."""
                    n = len(_alive_sbs(tci))
                    pvh = alloc_pvh(pool)
                    for tb in range(4):
                        emit_pv_group(pair, pvh, tes, tb, tci, n)
                        if tail_split and tb < 3:
                            # per-tb norm+DMA so the HWDGE queue is clear
                            # when the final block's store issues
                            emit_norm(pair, tci, pvh, tb, tb + 1,
                                      act_dma=True)
                    if tail_split:
                        emit_norm(pair, tci, pvh, 3, 4, dma_eng=nc.sync)
                    else:
                        emit_norm(pair, tci, pvh, 0, 4)

                def emit_pv_tail(pair, tci, tes):
                    """Tail-chunk PV: t-blocks spread over separate banks so
                    each block's norm+store flows without serializing the
                    next block's start=True (whole-bank pending-zero WAR)."""
                    n = len(_alive_sbs(tci))
                    ppA = ppps.tile([SB, TC], F32, tag="pp", name="pvtA")
                    ppB = ppps.tile([SB, TC], F32, tag="pp", name="pvtB")
                    pvC = pvps.tile([SB, 260], F32, tag="pv", name="pvtC")
                    phA = [(ppA, 0, TC), (ppA, 256, TC)]
                    phB = [(ppB, 0, TC), (ppB, 256, TC)]
                    phC = [(pvC, 0, 260), (pvC, 130, 260)]
                    emit_pv_group(pair, phA, tes, 0, tci, n, ltb=0)
                    emit_pv_group(pair, phA, tes, 1, tci, n, ltb=1)
                    emit_norm(pair, tci, phA, 0, 2, act_dma=True)
                    emit_pv_group(pair, phB, tes, 2, tci, n, ltb=0)
                    emit_norm(pair, tci, phB, 0, 1, act_dma=True, tb_off=2)
                    emit_pv_group(pair, phC, tes, 3, tci, n, ltb=0)
                    emit_norm(pair, tci, phC, 0, 1, dma_eng=nc.sync,
                              tb_off=3)

                def emit_special_probs():
                    """t=512 scores, column-major: psp[s, 4g+sb]; one exp."""
                    psp = ppps.tile([SB, TC], F32, tag="pp", name="psp")
                    for g in range(NHC):
                        pair, h2 = g // 2, g % 2
                        qcol = qt[pair][h2 * HD : (h2 + 1) * HD, 512:513]
                        for sb in range(4):
                            nc.tensor.matmul(
                                psp[:, 4 * g + sb : 4 * g + sb + 1],
                                kt[pair][h2 * HD : (h2 + 1) * HD,
                                         sb * SB : (sb + 1) * SB],
                                qcol,
                                start=True, stop=True,
                                skip_group_check=True,
                                tile_position=(h2 * HD, 0),
                            )
                        # j=512 tail term (partition 0)
                        nc.tensor.matmul(
                            psp[0:1, 16 + g : 17 + g],
                            kt[pair][h2 * HD : (h2 + 1) * HD, 512:513],
                            qcol,
                            start=True, stop=True,
                            skip_group_check=True,
                            tile_position=(h2 * HD, 0),
                        )
                    if with_attc:
                        am = bass_mod.AP(
                            tensor=attcs[:].tensor, offset=0,
                            ap=[[NSB, SB], [0, NHC], [1, 4]],
                        )
                        nc.vector.tensor_add(psp[:, 0:16], psp[:, 0:16], am)
                        am5 = bass_mod.AP(
                            tensor=attr_t[:].tensor, offset=512,
                            ap=[[0, 1], [0, NHC]],
                        )
                        nc.vector.tensor_add(
                            psp[0:1, 16:20], psp[0:1, 16:20], am5
                        )
                    nc.scalar.activation(
                        erT[:, 0:20], psp[:, 0:20], EXP, scale=SCALE
                    )

                def emit_special_pv():
                    """Row-major PV from the column-major probs in erT."""
                    pvs = pvps.tile([SB, 260], F32, tag="pv", name="pvs")
                    for g in range(NHC):
                        for sb4 in range(4):
                            nc.tensor.matmul(
                                pvs[0:1, g * (HD + 1) :
                                    (g + 1) * (HD + 1)],
                                erT[:, 4 * g + sb4 : 4 * g + sb4 + 1],
                                vt[sb4][:, g * (HD + 1) :
                                        (g + 1) * (HD + 1)],
                                start=(sb4 == 0),
                                stop=False,
                                skip_group_check=True,
                            )
                        nc.tensor.matmul(
                            pvs[0:1, g * (HD + 1) : (g + 1) * (HD + 1)],
                            erT[0:1, 16 + g : 17 + g],
                            vt[4][0:1, g * (HD + 1) : (g + 1) * (HD + 1)],
                            start=False, stop=True,
                            skip_group_check=True,
                        )
                    zsrc = bass_mod.AP(
                        tensor=pvs[:].tensor, offset=HD,
                        ap=[[260, 1], [HD + 1, NHC], [1, 1]],
                    )
                    nc.vector.reciprocal(rz4[0:1, 0:NHC], zsrc)
                    for g in range(NHC):
                        nc.vector.tensor_scalar_mul(
                            svn[0:1, g * HD : (g + 1) * HD],
                            pvs[0:1, g * (HD + 1) :
                                g * (HD + 1) + HD],
                            rz4[0:1, g : g + 1],
                        )
                    nc.sync.dma_start(out=out_d[512:513, :], in_=svn[:])

                # startup: q00/k00 only, k-major, riding the DMA pipeline.
                # v projections move to feed (their only consumers, the PVs,
                # are deferred) so the first exp lands ~5us earlier and ACT
                # saturates sooner.
                pre_qk = mmps.tile([SB, 2 * TC], F32, tag="mm", name="preqk")
                # k0 in t-halves (t 0:256 arrives in the first DMA piece)
                for half in range(2):
                    for col0 in (0, TC):
                        nc.tensor.matmul(
                            pre_qk[:, col0 + half * 256 :
                                   col0 + (half + 1) * 256],
                            hw[:, _wcol(0, col0 // 4) :
                               _wcol(0, col0 // 4) + SB],
                            hw[:, _hcol(0, half * 256) :
                               _hcol(0, half * 256) + 256],
                            start=(half == 0), stop=False,
                        )
                for k in range(1, NK):
                    nc.tensor.matmul(
                        pre_qk[:, 0:TC],
                        hw[:, _wcol(k, 0) : _wcol(k, 0) + SB],
                        hw[:, _hcol(k, 0) : _hcol(k, 0) + TC],
                        start=False, stop=(k == NKA - 1),
                    )
                    nc.tensor.matmul(
                        pre_qk[:, TC : 2 * TC],
                        hw[:, _wcol(k, SB) : _wcol(k, SB) + SB],
                        hw[:, _hcol(k, 0) : _hcol(k, 0) + TC],
                        start=False, stop=(k == NKA - 1),
                    )
                if with_bias:
                    nc.tensor.matmul(
                        pre_qk[:, 0:TC], w9[:, 0:SB], hst9[:, 0:TC],
                        start=False, stop=True,
                    )
                    nc.tensor.matmul(
                        pre_qk[:, TC : 2 * TC], w9[:, SB : 2 * SB],
                        hst9[:, 0:TC], start=False, stop=True,
                    )
                # piecewise copies, descending halves, so L00's QK
                # (descending sb order) starts as soon as its slice lands;
                # qt halves go via the idle GPSIMD engine in parallel
                for piece in (1, 0):
                    nc.vector.tensor_copy(
                        kt[0][:, piece * 256 : (piece + 1) * 256],
                        pre_qk[:, TC + piece * 256 : TC + (piece + 1) * 256])
                    nc.vector.tensor_copy(
                        qt[0][:, piece * 256 : (piece + 1) * 256],
                        pre_qk[:, piece * 256 : (piece + 1) * 256])

                # v7 schedule: QK of every chunk as early as deps allow so
                # ACT (exp) saturates from ~10us on; all PV work floats to
                # wherever PE has slack vs ACT, finishing with a pure-PE
                # endgame that runs while ACT drains the last exps.
                q01 = thunks_q(0, 1)
                v0 = thunks_v(0)
                te00 = emit_attn(0, 0, feed=v0[:5] + q01[:4]
                                 + v0[5:] + q01[4:],
                                 sb_order=[3, 2, 1, 0])
                te01 = emit_attn(0, 1, feed=thunks_k(0, 1) + thunks_q(0, 2))
                te02 = emit_attn(0, 2, feed=thunks_v(1) + thunks_q(1, 0)
                                 + thunks_k(1, 0))
                te10 = emit_attn(1, 0, feed=thunks_q(1, 1) + thunks_k(1, 1))
                te11 = emit_attn(1, 1, feed=thunks_q(0, 3) + thunks_k(0, 2))
                emit_special_probs()
                te03 = emit_attn(
                    0, 3,
                    feed=thunks_q(1, 2) + thunks_v(2) + thunks_v(3)
                    + thunks_v(4) + thunks_v(5) + thunks_v(6)
                    + thunks_v(7))
                q13 = thunks_q(1, 3)
                k12 = thunks_k(1, 2)
                te12 = emit_attn(
                    1, 2,
                    feed=q13[:5] + [lambda: emit_pv_chunk(0, 0, te00)]
                    + q13[5:] + [lambda: emit_pv_chunk(0, 1, te01)]
                    + k12[:5] + [lambda: emit_pv_chunk(1, 0, te10)]
                    + k12[5:] + [lambda: emit_pv_chunk(1, 1, te11)])
                te13 = emit_attn(
                    1, 3,
                    feed=thunks_v(8)
                    + [lambda: emit_pv_chunk(0, 2, te02)]
                    + thunks_v(9)
                    + [lambda: emit_pv_chunk(1, 2, te12)]
                    + thunks_v(10) + [emit_special_pv]
                    + thunks_v(11)
                    + [lambda: emit_pv_chunk(0, 3, te03)])
                # endgame: only the last chunk's PV remains after the final
                # exps; its accumulators borrow the idle projection pool so
                # nothing serializes on pvps WARs
                emit_pv_tail(1, 3, te13)

    nc.compile()
    return nc


def _host_prep(inputs, with_bias, with_attc):
    import ml_dtypes

    bf = ml_dtypes.bfloat16
    hs = np.asarray(inputs["hidden_states"], dtype=np.float32)
    am = np.asarray(inputs["attention_mask"], dtype=np.float32)
    Wq = np.asarray(inputs["Wq"], dtype=np.float32)
    bq = np.asarray(inputs["bq"], dtype=np.float32)
    Wk = np.asarray(inputs["Wk"], dtype=np.float32)
    bk = np.asarray(inputs["bk"], dtype=np.float32)
    Wv = np.asarray(inputs["Wv"], dtype=np.float32)
    bv = np.asarray(inputs["bv"], dtype=np.float32)

    p = np.arange(SB)[:, None]
    x = np.arange(SB)[None, :]
    m0 = (p <= x).astype(np.float32)
    m1 = (p <= x - 1).astype(np.float32)
    ones = np.ones((SB, 1), dtype=np.float32)
    masks = np.concatenate([m0, m1, ones], axis=1).astype(bf)

    in_maps = []
    for c in range(8):
        b, g = c // 4, c % 4
        # hst [128, k, 2048]: hst[p, k, t] = hs[b, t, 128k+p]
        hsT = hs[b].T  # [1024, 2048]
        hstp = hsT.reshape(NK, SB, S).transpose(1, 0, 2)
        # w [128, k, 768]: [q0|k0|q1|k1|v], w[p,k,256e+j]=W[256g+128e+j,128k+p]
        w = np.zeros((SB, NK, WK), dtype=np.float32)
        Wq_sl = Wq[256 * g : 256 * (g + 1), :]  # [256, 1024]
        Wk_sl = Wk[256 * g : 256 * (g + 1), :]
        Wv_sl = Wv[256 * g : 256 * (g + 1), :]
        for k in range(NK):
            cols = slice(SB * k, SB * (k + 1))
            w[:, k, 0:128] = Wq_sl[0:128, cols].T
            w[:, k, 128:256] = Wk_sl[0:128, cols].T
            w[:, k, 256:384] = Wq_sl[128:256, cols].T
            w[:, k, 384:512] = Wk_sl[128:256, cols].T
            w[:, k, 512:768] = Wv_sl[:, cols].T
        # interleave into need-ordered sections (see _wcol/_hcol)
        hwm = np.zeros((SB, NK, SEC), dtype=np.float32)
        hwm[:, :, 0:256] = w[:, :, 0:256]
        hwm[:, :, 256:768] = hstp[:, :, 0:512]
        hwm[:, :, 768:1024] = w[:, :, 512:768]
        hwm[:, :, 1024:1280] = w[:, :, 256:512]
        hwm[:, :, 1280:1792] = hstp[:, :, 512:1024]
        hwm[:, :, 1792:2304] = hstp[:, :, 1024:1536]
        hwm[:, :, 2304:2816] = hstp[:, :, 1536:2048]
        amv = am[b, 0, 0, :].astype(np.float32)
        attc = np.ascontiguousarray(amv.reshape(NSB, SB).T)
        m = {
            "hw": hwm.reshape(SB, NK * SEC).astype(bf),
            "attc": attc,
            "masks": masks.copy(),
        }
        if with_attc:
            m["attr"] = (amv[:513] / SCALE).reshape(1, 513).copy()
            m["attcs"] = (attc / SCALE).copy()
        if with_bias:
            bsl = np.zeros((WK,), dtype=np.float32)
            bsl[0:128] = bq[256 * g : 256 * g + 128]
            bsl[128:256] = bk[256 * g : 256 * g + 128]
            bsl[256:384] = bq[256 * g + 128 : 256 * g + 256]
            bsl[384:512] = bk[256 * g + 128 : 256 * g + 256]
            bsl[512:768] = bv[256 * g : 256 * (g + 1)]
            m["hst9"] = np.ones((1, S), dtype=np.float32).astype(bf)
            m["w9"] = bsl.reshape(1, WK).astype(bf)
        in_maps.append(m)
    return in_maps


LAST_EXEC_NS = None


def kernel(**inputs):
    import os

    from concourse.bass_utils import run_bass_kernel_spmd

    global LAST_EXEC_NS
    with_bias = bool(
        np.any(np.asarray(inputs["bq"]))
        or np.any(np.asarray(inputs["bk"]))
        or np.any(np.asarray(inputs["bv"]))
    )
    with_attc = bool(np.any(np.asarray(inputs["attention_mask"])))
    key = f"nc{int(with_bias)}{int(with_attc)}"
    if key not in _CACHE:
        _CACHE[key] = _build_program(with_bias=with_bias,
                                     with_attc=with_attc)
    nc = _CACHE[key]
    in_maps = _host_prep(inputs, with_bias, with_attc)
    trace = bool(os.environ.get("BASS_KERNEL_TRACE"))
    res = run_bass_kernel_spmd(nc, in_maps, list(range(8)), trace=trace)
    LAST_EXEC_NS = res.exec_time_ns
    out = np.empty((B, S, H), dtype=np.float32)
    for c in range(8):
        b, g = c // 4, c % 4
        out[b, :, 256 * g : 256 * (g + 1)] = np.asarray(
            res.results[c]["out"]).astype(np.float32)
    return out
